# revision 11
# baseline (speedup 1.0000x reference)
"""Trainium2 Bass kernel for nn_AutoCorrelation (multi-head attention with a
distance decay bias), SPMD across 8 NeuronCores.

Sharding: core = (batch b, head-group hg) with b in 0..3, hg in 0..1.
Each core computes, for its batch and its 8 heads: QKV projections
(column-sharded weights), distance-banded attention (the -0.1*|i-j| bias makes
weights beyond |i-j|=256 numerically zero), and a row-sharded output
projection. The host sums the two half partial outputs per batch and adds the
effective output bias.

Math notes:
 - bk drops out entirely (softmax row-shift invariance: K-side bias only adds
   per-query constants to scores).
 - bv passes through attention (softmax rows sum to 1) and is folded into the
   host-side output bias: bo_eff = bo + Wo @ bv.
 - scores are built transposed St[k, q] so the P@V matmul needs no transposes;
   a ones-column appended to V yields the softmax denominators in the same
   matmul (PSUM row 64).
 - the bias exp(-0.1|k-q|) is a Toeplitz multiply: one [128, 1280] master
   array is precomputed on host and sliced per k-chunk.
"""

import math
from contextlib import ExitStack

import numpy as np
import ml_dtypes

BF16 = ml_dtypes.bfloat16

N_CORES = 8


class Cfg:
    def __init__(self, L=2048, C=1024, NHL=8, DK=64, W=256):
        self.L, self.C, self.NHL, self.DK, self.W = L, C, NHL, DK, W
        self.DL = NHL * DK               # local head dims
        self.SPAN = 128 + 2 * W          # k-chunk q-span
        self.KC = L // 128               # k chunks
        self.NQB = L // 128              # q blocks (128)
        self.NQT = L // 512              # q tiles (512)
        self.CC = C // 128               # contraction chunks
        self.LT = L // 512               # l tiles
        self.HP = NHL // 2               # head pairs
        self.VW = NHL * 65               # padded V width
        self.EBW = self.SPAN + 512       # EB master width
        assert self.SPAN % 128 == 0 and self.SPAN <= L

    def qs_of(self, kc):
        return min(max(128 * kc - self.W, 0), self.L - self.SPAN)

    def covers(self, qb):
        """k-chunks whose span fully covers q-block qb (spans are 128-aligned)."""
        return [kc for kc in range(self.KC)
                if self.qs_of(kc) <= 128 * qb and self.qs_of(kc) + self.SPAN >= 128 * (qb + 1)]


FULL = Cfg()


def build_program(cfg=FULL, debug=False):
    import concourse.bass as bass
    import concourse.tile as tile
    from concourse import bacc, mybir

    f32 = mybir.dt.float32
    bf16 = mybir.dt.bfloat16
    AF = mybir.ActivationFunctionType

    L, C, NHL, DL, W = cfg.L, cfg.C, cfg.NHL, cfg.DL, cfg.W
    SPAN, KC, NQT, CC, LT, HP, VW = cfg.SPAN, cfg.KC, cfg.NQT, cfg.CC, cfg.LT, cfg.HP, cfg.VW

    nc = bacc.Bacc("TRN2", target_bir_lowering=False, debug=debug,
                   num_devices=N_CORES)

    xq = nc.dram_tensor("xq", [C, L], bf16, kind="ExternalInput").ap()
    xk = nc.dram_tensor("xk", [C, L], bf16, kind="ExternalInput").ap()
    xv = nc.dram_tensor("xv", [C, L], bf16, kind="ExternalInput").ap()
    wq = nc.dram_tensor("wq", [C, DL], bf16, kind="ExternalInput").ap()
    wk = nc.dram_tensor("wk", [C, DL], bf16, kind="ExternalInput").ap()
    wv = nc.dram_tensor("wv", [C, VW], bf16, kind="ExternalInput").ap()
    wo = nc.dram_tensor("wo", [DL, C], bf16, kind="ExternalInput").ap()
    bqd = nc.dram_tensor("bq", [DL, 1], f32, kind="ExternalInput").ap()
    ebd = nc.dram_tensor("eb", [128, cfg.EBW], bf16, kind="ExternalInput").ap()
    erow = nc.dram_tensor("erow", [1, VW], bf16, kind="ExternalInput").ap()
    onesr = nc.dram_tensor("onesr", [1, 128], bf16, kind="ExternalInput").ap()
    out = nc.dram_tensor("out", [L, C], f32, kind="ExternalOutput").ap()

    def nsplit(total, cap=512):
        o, r = [], 0
        while r < total:
            n = min(cap, total - r)
            o.append((r, n))
            r += n
        return o

    with tile.TileContext(nc) as tc, ExitStack() as ctx:
        const = ctx.enter_context(tc.tile_pool(name="const", bufs=1))
        big = ctx.enter_context(tc.tile_pool(name="big", bufs=1))
        xs = ctx.enter_context(tc.tile_pool(name="xs", bufs=2))
        ets = ctx.enter_context(tc.tile_pool(name="ets", bufs=3))
        small = ctx.enter_context(tc.tile_pool(name="small", bufs=1))
        rbp = ctx.enter_context(tc.tile_pool(name="rbp", bufs=2))
        ostage = ctx.enter_context(tc.tile_pool(name="ostage", bufs=3))
        psum = ctx.enter_context(tc.tile_pool(name="psum", bufs=1, space="PSUM"))

        # ---- resident constants ----
        wq_sb = const.tile([128, CC * DL], bf16)
        wk_sb = const.tile([128, CC * DL], bf16)
        wv_sb = const.tile([128, CC * VW], bf16)
        wo_sb = const.tile([128, HP * C], bf16)
        for c in range(CC):
            nc.sync.dma_start(wq_sb[:, c * DL:(c + 1) * DL], wq[c * 128:(c + 1) * 128, :])
            nc.sync.dma_start(wk_sb[:, c * DL:(c + 1) * DL], wk[c * 128:(c + 1) * 128, :])
            nc.sync.dma_start(wv_sb[:, c * VW:(c + 1) * VW], wv[c * 128:(c + 1) * 128, :])
        for hp in range(HP):
            nc.sync.dma_start(wo_sb[:, hp * C:(hp + 1) * C], wo[hp * 128:(hp + 1) * 128, :])
        eb_sb = const.tile([128, cfg.EBW], bf16)
        nc.sync.dma_start(eb_sb[:], ebd[:])
        erow_sb = const.tile([1, VW], bf16)
        nc.sync.dma_start(erow_sb[:], erow[:])
        onesr_sb = const.tile([1, 128], bf16)
        nc.sync.dma_start(onesr_sb[:], onesr[:])
        bq_sb = const.tile([128, HP], f32)
        for hp in range(HP):
            nc.sync.dma_start(bq_sb[:, hp:hp + 1], bqd[hp * 128:(hp + 1) * 128, :])

        # ---- resident activations ----
        qt_sb = [big.tile([128, L], bf16, name=f"qt{hp}") for hp in range(HP)]
        kt_sb = [big.tile([128, L], bf16, name=f"kt{hp}") for hp in range(HP)]
        vb_sb = big.tile([128, KC * VW], bf16)
        oraw_sb = [big.tile([128, L], bf16, name=f"oraw{hp}") for hp in range(HP)]
        ots_sb = [big.tile([128, L], bf16, name=f"ots{hp}") for hp in range(HP)]
        s_sb = [small.tile([NHL, 512], f32, name=f"s{qt}") for qt in range(NQT)]
        stage = ctx.enter_context(tc.tile_pool(name="stage", bufs=4))

        # ================= Phase A: projections =================
        for lt in range(LT):
            for which, xdram in (("q", xq), ("k", xk), ("v", xv)):
                x_sb = xs.tile([128, CC * 512], bf16, tag="xs", name=f"x_{which}{lt}")
                for c in range(CC):
                    nc.sync.dma_start(
                        x_sb[:, c * 512:(c + 1) * 512],
                        xdram[c * 128:(c + 1) * 128, lt * 512:(lt + 1) * 512])
                if which in ("q", "k"):
                    w_sb = wq_sb if which == "q" else wk_sb
                    t_sb = qt_sb if which == "q" else kt_sb
                    for hp in range(HP):
                        ps = psum.tile([128, 512], f32, tag="one", bufs=4,
                                       name=f"psp_{which}{lt}_{hp}")
                        for c in range(CC):
                            nc.tensor.matmul(
                                ps[:],
                                lhsT=w_sb[:, c * DL + hp * 128: c * DL + hp * 128 + 128],
                                rhs=x_sb[:, c * 512:(c + 1) * 512],
                                start=(c == 0), stop=(c == CC - 1))
                        dst = t_sb[hp][:, lt * 512:(lt + 1) * 512]
                        if which == "q":
                            nc.scalar.activation(dst, ps[:], AF.Identity,
                                                 bias=bq_sb[:, hp:hp + 1], scale=1.0)
                        else:
                            nc.vector.tensor_copy(dst, ps[:])
                else:
                    for sub in range(4):
                        kcg = lt * 4 + sub
                        ps = psum.tile([128, SPAN], f32, tag="two", bufs=2,
                                       name=f"psp_v{kcg}")
                        pieces = nsplit(VW)
                        for c in range(CC):
                            lhsT = x_sb[:, c * 512 + sub * 128: c * 512 + sub * 128 + 128]
                            for (o, n) in pieces:
                                nc.tensor.matmul(
                                    ps[:, o:o + n], lhsT=lhsT,
                                    rhs=wv_sb[:, c * VW + o: c * VW + o + n],
                                    start=(c == 0), stop=False)
                        for (o, n) in pieces:
                            nc.tensor.matmul(
                                ps[:, o:o + n], lhsT=onesr_sb[0:1, :],
                                rhs=erow_sb[0:1, o:o + n], start=False, stop=True)
                        nc.vector.tensor_copy(
                            vb_sb[:, kcg * VW:(kcg + 1) * VW], ps[:, 0:VW])

        # ================= Phase B: banded attention =================
        # For each q-tile (one PSUM bank): the ordered (kc, j) matmuls that hit
        # it. PSUM start marks the whole bank pending-zero, so only the very
        # first MM into a bank gets start=True and only the very last gets
        # stop=True; intermediate first-touches of other columns overwrite
        # their pending-zero bytes.
        qt_mms = {qt: [] for qt in range(NQT)}
        for kc in range(KC):
            for j in range(SPAN // 128):
                qb = cfg.qs_of(kc) // 128 + j
                qt_mms[qb // 4].append((kc, j))
        qt_first = {qt: mms[0] for qt, mms in qt_mms.items()}
        qt_last = {qt: mms[-1] for qt, mms in qt_mms.items()}
        qt_done_at = {qt: mms[-1][0] for qt, mms in qt_mms.items()}

        for h in range(NHL):
            hp, hi = h // 2, h % 2
            po = {}
            for kc in range(KC):
                qs = cfg.qs_of(kc)
                seb = qs - 128 * kc + 512
                ps = psum.tile([128, SPAN], f32, tag="two", bufs=2,
                               name=f"ps_s{h}_{kc}")
                lhsT = kt_sb[hp][hi * 64:(hi + 1) * 64, kc * 128:(kc + 1) * 128]
                for (o, n) in nsplit(SPAN):
                    nc.tensor.matmul(
                        ps[:, o:o + n], lhsT=lhsT,
                        rhs=qt_sb[hp][hi * 64:(hi + 1) * 64, qs + o: qs + o + n],
                        start=True, stop=True)
                et = ets.tile([128, SPAN], bf16, tag="et", name=f"et{h}_{kc}")
                nc.scalar.activation(et[:], ps[:], AF.Exp, scale=0.125)
                etb = ets.tile([128, SPAN], bf16, tag="etb", name=f"etb{h}_{kc}")
                nc.vector.tensor_mul(etb[:], et[:], eb_sb[:, seb:seb + SPAN])
                vsl = vb_sb[:, kc * VW + h * 65: kc * VW + h * 65 + 65]
                for j in range(SPAN // 128):
                    qb = qs // 128 + j
                    qt_i = qb // 4
                    qoff = (qb % 4) * 128
                    if qt_i not in po:
                        po[qt_i] = psum.tile([128, 512], f32, tag="one", bufs=4,
                                             name=f"po{h}_{qt_i}")
                    nc.tensor.matmul(
                        po[qt_i][0:65, qoff:qoff + 128], lhsT=vsl,
                        rhs=etb[:, j * 128:(j + 1) * 128],
                        start=(qt_first[qt_i] == (kc, j)),
                        stop=(qt_last[qt_i] == (kc, j)))
                for qt_i in [q for q, t in po.items() if qt_done_at[q] == kc]:
                    t = po.pop(qt_i)
                    # engines address partitions in 32-strips only: stage the s
                    # row at partition 0, then DMA it to partition h
                    s_st = stage.tile([1, 512], f32, tag="ss", name=f"ss{h}_{qt_i}")
                    nc.scalar.copy(s_st[:], t[64:65, :])
                    nc.sync.dma_start(s_sb[qt_i][h:h + 1, :], s_st[:])
                    nc.vector.tensor_copy(
                        oraw_sb[hp][hi * 64:(hi + 1) * 64, qt_i * 512:(qt_i + 1) * 512],
                        t[0:64, :])

        # ---- normalization ----
        for qt in range(NQT):
            r_f = stage.tile([NHL, 512], f32, tag="rf", name=f"rf{qt}")
            nc.vector.reciprocal(r_f[:], s_sb[qt][:])
            r_b = stage.tile([NHL, 512], bf16, tag="rb", name=f"rb{qt}")
            nc.vector.tensor_copy(r_b[:], r_f[:])
            for h in range(NHL):
                hp, hi = h // 2, h % 2
                r_st = stage.tile([1, 512], bf16, tag="rs", name=f"rs{qt}_{h}")
                nc.sync.dma_start(r_st[:], r_b[h:h + 1, :])
                # DVE tensor_tensor needs both SBUF inputs at the same base
                # partition: broadcast r to all 128 and slice the matching half
                rbb = rbp.tile([128, 512], bf16, tag="rbb", name=f"rbb{qt}_{h}")
                nc.gpsimd.partition_broadcast(rbb[:], r_st[:])
                sl = (slice(hi * 64, (hi + 1) * 64), slice(qt * 512, (qt + 1) * 512))
                nc.vector.tensor_mul(ots_sb[hp][sl], oraw_sb[hp][sl], rbb[sl[0], :])

        # ================= Phase C: output projection =================
        for qc in range(L // 128):
            for (mo, mn) in nsplit(C):
                pf = psum.tile([128, 512], f32, tag="one", bufs=4,
                               name=f"pf{qc}_{mo}")
                for hp in range(HP):
                    nc.tensor.matmul(
                        pf[:, 0:mn],
                        lhsT=ots_sb[hp][:, qc * 128:(qc + 1) * 128],
                        rhs=wo_sb[:, hp * C + mo: hp * C + mo + mn],
                        start=(hp == 0), stop=(hp == HP - 1))
                st = ostage.tile([128, 512], f32, tag="fo", name=f"fo{qc}_{mo}")
                nc.scalar.copy(st[:, 0:mn], pf[:, 0:mn])
                nc.sync.dma_start(out[qc * 128:(qc + 1) * 128, mo:mo + mn],
                                  st[:, 0:mn])

    nc.compile()
    return nc


def host_inputs(inputs, cfg=FULL):
    """Build the 8 per-core input maps + the host-side combine constant."""
    L, C, DL, NHL = cfg.L, cfg.C, cfg.DL, cfg.NHL
    q = np.asarray(inputs["queries"], np.float32)
    k = np.asarray(inputs["keys"], np.float32)
    v = np.asarray(inputs["values"], np.float32)
    Wq = np.asarray(inputs["Wq"], np.float32)
    Wk = np.asarray(inputs["Wk"], np.float32)
    Wv = np.asarray(inputs["Wv"], np.float32)
    Wo = np.asarray(inputs["Wo"], np.float32)
    bq = np.asarray(inputs["bq"], np.float32)
    bv = np.asarray(inputs["bv"], np.float32)
    bo = np.asarray(inputs["bo"], np.float32)
    B = q.shape[0]

    bo_eff = (bo.astype(np.float64) + Wo.astype(np.float64) @ bv.astype(np.float64)
              ).astype(np.float32)

    p = np.arange(128, dtype=np.float64)[:, None]
    c = np.arange(cfg.EBW, dtype=np.float64)[None, :]
    eb = np.exp(-0.1 * np.abs(p - c + 512)).astype(BF16)

    erow = np.zeros((1, cfg.VW), BF16)
    erow[0, 64::65] = 1.0
    onesr = np.ones((1, 128), BF16)

    xT = {}
    for b in range(B):
        xT[b] = (np.ascontiguousarray(q[b].T).astype(BF16),
                 np.ascontiguousarray(k[b].T).astype(BF16),
                 np.ascontiguousarray(v[b].T).astype(BF16))

    in_maps = []
    for core in range(N_CORES):
        b, hg = core // 2, core % 2
        sl = slice(hg * DL, (hg + 1) * DL)
        wvp = np.zeros((C, cfg.VW), np.float32)
        for h in range(NHL):
            wvp[:, h * 65:h * 65 + 64] = Wv.T[:, hg * DL + h * 64: hg * DL + (h + 1) * 64]
        in_maps.append({
            "xq": xT[b][0], "xk": xT[b][1], "xv": xT[b][2],
            "wq": np.ascontiguousarray(Wq.T[:, sl]).astype(BF16),
            "wk": np.ascontiguousarray(Wk.T[:, sl]).astype(BF16),
            "wv": wvp.astype(BF16),
            "wo": np.ascontiguousarray(Wo.T[sl, :]).astype(BF16),
            "bq": np.ascontiguousarray(bq[sl][:, None]),
            "eb": eb, "erow": erow, "onesr": onesr,
        })
    return in_maps, bo_eff


_CACHED = {}


def kernel(**inputs):
    from concourse.bass_utils import run_bass_kernel_spmd

    cfg = FULL
    if "nc" not in _CACHED:
        _CACHED["nc"] = build_program(cfg)
    nc = _CACHED["nc"]

    in_maps, bo_eff = host_inputs(inputs, cfg)
    res = run_bass_kernel_spmd(nc, in_maps, core_ids=list(range(N_CORES)))
    B = np.asarray(inputs["queries"]).shape[0]
    out = np.zeros((B, cfg.L, cfg.C), np.float32)
    for b in range(B):
        out[b] = (res.results[2 * b]["out"] + res.results[2 * b + 1]["out"]
                  + bo_eff[None, :])
    return out


# revision 18
# speedup vs baseline: 1.2950x; 1.2950x over previous
"""Trainium2 Bass kernel for nn_AutoCorrelation (multi-head attention with a
distance decay bias), SPMD across 8 NeuronCores.

Sharding: core = (batch b, head-group hg) with b in 0..3, hg in 0..1.
Each core computes, for its batch and its 8 heads: QKV projections
(column-sharded weights), distance-banded attention (the -0.1*|i-j| bias makes
weights beyond |i-j|=256 numerically zero), and a row-sharded output
projection. The host sums the two half partial outputs per batch and adds the
effective output bias.

Math notes:
 - bk drops out entirely (softmax row-shift invariance: K-side bias only adds
   per-query constants to scores).
 - bv passes through attention (softmax rows sum to 1) and is folded into the
   host-side output bias: bo_eff = bo + Wo @ bv.
 - scores are built transposed St[k, q] so the P@V matmul needs no transposes;
   a ones-column appended to V yields the softmax denominators in the same
   matmul (PSUM row 64).
 - the bias exp(-0.1|k-q|) is a Toeplitz multiply: one [128, 1280] master
   array is precomputed on host and sliced per k-chunk.
"""

import math
from contextlib import ExitStack

import numpy as np
import ml_dtypes

BF16 = ml_dtypes.bfloat16

N_CORES = 8


class Cfg:
    def __init__(self, L=2048, C=1024, NHL=8, DK=64, W=256):
        self.L, self.C, self.NHL, self.DK, self.W = L, C, NHL, DK, W
        self.DL = NHL * DK               # local head dims
        self.SPAN = 128 + 2 * W          # k-chunk q-span
        self.KC = L // 128               # k chunks
        self.NQB = L // 128              # q blocks (128)
        self.NQT = L // 512              # q tiles (512)
        self.CC = C // 128               # contraction chunks
        self.LT = L // 512               # l tiles
        self.HP = NHL // 2               # head pairs
        self.VW = NHL * 65               # padded V width
        self.EBW = self.SPAN + 512       # EB master width
        assert self.SPAN % 128 == 0 and self.SPAN <= L

    def qs_of(self, kc):
        return min(max(128 * kc - self.W, 0), self.L - self.SPAN)

    def covers(self, qb):
        """k-chunks whose span fully covers q-block qb (spans are 128-aligned)."""
        return [kc for kc in range(self.KC)
                if self.qs_of(kc) <= 128 * qb and self.qs_of(kc) + self.SPAN >= 128 * (qb + 1)]


FULL = Cfg(W=128)


def build_program(cfg=FULL, debug=False):
    import concourse.bass as bass
    import concourse.tile as tile
    from concourse import bacc, mybir

    f32 = mybir.dt.float32
    bf16 = mybir.dt.bfloat16
    AF = mybir.ActivationFunctionType

    L, C, NHL, DL, W = cfg.L, cfg.C, cfg.NHL, cfg.DL, cfg.W
    SPAN, KC, NQT, CC, LT, HP, VW = cfg.SPAN, cfg.KC, cfg.NQT, cfg.CC, cfg.LT, cfg.HP, cfg.VW

    nc = bacc.Bacc("TRN2", target_bir_lowering=False, debug=debug,
                   num_devices=N_CORES)

    xq = nc.dram_tensor("xq", [C, L], bf16, kind="ExternalInput").ap()
    xk = nc.dram_tensor("xk", [C, L], bf16, kind="ExternalInput").ap()
    xv = nc.dram_tensor("xv", [C, L], bf16, kind="ExternalInput").ap()
    wq = nc.dram_tensor("wq", [C, DL], bf16, kind="ExternalInput").ap()
    wk = nc.dram_tensor("wk", [C, DL], bf16, kind="ExternalInput").ap()
    wv = nc.dram_tensor("wv", [C, VW], bf16, kind="ExternalInput").ap()
    wo = nc.dram_tensor("wo", [DL, C], bf16, kind="ExternalInput").ap()
    bqd = nc.dram_tensor("bq", [DL, 1], f32, kind="ExternalInput").ap()
    ebd = nc.dram_tensor("eb", [128, cfg.EBW], bf16, kind="ExternalInput").ap()
    erow = nc.dram_tensor("erow", [1, VW], bf16, kind="ExternalInput").ap()
    onesr = nc.dram_tensor("onesr", [1, 128], bf16, kind="ExternalInput").ap()
    out = nc.dram_tensor("out", [L, C], f32, kind="ExternalOutput").ap()

    def nsplit(total, cap=512):
        o, r = [], 0
        while r < total:
            n = min(cap, total - r)
            o.append((r, n))
            r += n
        return o

    with tile.TileContext(nc) as tc, ExitStack() as ctx:
        const = ctx.enter_context(tc.tile_pool(name="const", bufs=1))
        big = ctx.enter_context(tc.tile_pool(name="big", bufs=1))
        xs = ctx.enter_context(tc.tile_pool(name="xs", bufs=2))
        ets = ctx.enter_context(tc.tile_pool(name="ets", bufs=3))
        rbp = ctx.enter_context(tc.tile_pool(name="rbp", bufs=2))
        ostage = ctx.enter_context(tc.tile_pool(name="ostage", bufs=3))
        psum = ctx.enter_context(tc.tile_pool(name="psum", bufs=1, space="PSUM"))

        # ---- resident constants ----
        wq_sb = const.tile([128, CC * DL], bf16)
        wk_sb = const.tile([128, CC * DL], bf16)
        wv_sb = const.tile([128, CC * VW], bf16)
        wo_sb = const.tile([128, HP * C], bf16)
        for c in range(CC):
            nc.sync.dma_start(wq_sb[:, c * DL:(c + 1) * DL], wq[c * 128:(c + 1) * 128, :])
            nc.sync.dma_start(wk_sb[:, c * DL:(c + 1) * DL], wk[c * 128:(c + 1) * 128, :])
            nc.sync.dma_start(wv_sb[:, c * VW:(c + 1) * VW], wv[c * 128:(c + 1) * 128, :])
        for hp in range(HP):
            nc.sync.dma_start(wo_sb[:, hp * C:(hp + 1) * C], wo[hp * 128:(hp + 1) * 128, :])
        eb_sb = const.tile([128, cfg.EBW], bf16)
        nc.sync.dma_start(eb_sb[:], ebd[:])
        erow_sb = const.tile([1, VW], bf16)
        nc.sync.dma_start(erow_sb[:], erow[:])
        onesr_sb = const.tile([1, 128], bf16)
        nc.sync.dma_start(onesr_sb[:], onesr[:])
        bq_sb = const.tile([128, HP], f32)
        for hp in range(HP):
            nc.sync.dma_start(bq_sb[:, hp:hp + 1], bqd[hp * 128:(hp + 1) * 128, :])

        # ---- resident activations ----
        qt_sb = [big.tile([128, L], bf16, name=f"qt{hp}") for hp in range(HP)]
        kt_sb = [big.tile([128, L], bf16, name=f"kt{hp}") for hp in range(HP)]
        vb_sb = big.tile([128, KC * VW], bf16)
        oraw_sb = [big.tile([128, L], bf16, name=f"oraw{hp}") for hp in range(HP)]
        ots_sb = [big.tile([128, L], bf16, name=f"ots{hp}") for hp in range(HP)]
        stage = ctx.enter_context(tc.tile_pool(name="stage", bufs=4))

        # ================= Phase A: projections =================
        for lt in range(LT):
            for which, xdram in (("q", xq), ("k", xk), ("v", xv)):
                x_sb = xs.tile([128, CC * 512], bf16, tag="xs", name=f"x_{which}{lt}")
                for c in range(CC):
                    nc.sync.dma_start(
                        x_sb[:, c * 512:(c + 1) * 512],
                        xdram[c * 128:(c + 1) * 128, lt * 512:(lt + 1) * 512])
                if which in ("q", "k"):
                    w_sb = wq_sb if which == "q" else wk_sb
                    t_sb = qt_sb if which == "q" else kt_sb
                    for hp in range(HP):
                        ps = psum.tile([128, 512], f32, tag="one", bufs=4,
                                       name=f"psp_{which}{lt}_{hp}")
                        for c in range(CC):
                            nc.tensor.matmul(
                                ps[:],
                                lhsT=w_sb[:, c * DL + hp * 128: c * DL + hp * 128 + 128],
                                rhs=x_sb[:, c * 512:(c + 1) * 512],
                                start=(c == 0), stop=(c == CC - 1))
                        dst = t_sb[hp][:, lt * 512:(lt + 1) * 512]
                        if which == "q":
                            nc.scalar.activation(dst, ps[:], AF.Identity,
                                                 bias=bq_sb[:, hp:hp + 1], scale=1.0)
                        else:
                            nc.vector.tensor_copy(dst, ps[:])
                else:
                    for sub in range(4):
                        kcg = lt * 4 + sub
                        pieces = nsplit(VW)
                        pss = [psum.tile([128, n], f32, tag="one", bufs=4,
                                         name=f"psp_v{kcg}_{o}") for (o, n) in pieces]
                        for c in range(CC):
                            lhsT = x_sb[:, c * 512 + sub * 128: c * 512 + sub * 128 + 128]
                            for pi, (o, n) in enumerate(pieces):
                                nc.tensor.matmul(
                                    pss[pi][:], lhsT=lhsT,
                                    rhs=wv_sb[:, c * VW + o: c * VW + o + n],
                                    start=(c == 0), stop=False)
                        for pi, (o, n) in enumerate(pieces):
                            nc.tensor.matmul(
                                pss[pi][:], lhsT=onesr_sb[0:1, :],
                                rhs=erow_sb[0:1, o:o + n], start=False, stop=True)
                            nc.vector.tensor_copy(
                                vb_sb[:, kcg * VW + o: kcg * VW + o + n], pss[pi][:])

        # ================= Phase B: banded attention =================
        # Merged PV matmul descriptors: per kc, contiguous 128-blocks hitting
        # the same q-tile are fused into one MM. PSUM start marks the whole
        # bank pending-zero, so only the very first MM into a po bank gets
        # start=True and only the very last gets stop=True.
        first_touch = {qb: min(cfg.covers(qb)) for qb in range(cfg.NQB)}
        pv_mms = []          # (kc, qt_i, qoff, ncols, etb_off)
        qt_order = {qt: [] for qt in range(NQT)}
        for kc in range(KC):
            qs = cfg.qs_of(kc)
            qbs = [qs // 128 + j for j in range(SPAN // 128)]
            # split runs at q-tile boundaries AND at first-touch boundaries so
            # each MM's PSUM range is uniformly pending-zero or accumulating
            run = [qbs[0]]
            for qb in qbs[1:] + [None]:
                if (qb is not None and qb // 4 == run[0] // 4
                        and (first_touch[qb] == kc) == (first_touch[run[0]] == kc)):
                    run.append(qb)
                else:
                    qt_i = run[0] // 4
                    mm_id = len(pv_mms)
                    pv_mms.append((kc, qt_i, (run[0] % 4) * 128, len(run) * 128,
                                   (run[0] - qs // 128) * 128))
                    qt_order[qt_i].append(mm_id)
                    run = [qb] if qb is not None else []
        qt_first = {qt: ids[0] for qt, ids in qt_order.items()}
        qt_last = {qt: ids[-1] for qt, ids in qt_order.items()}
        qt_done_at = {qt: pv_mms[ids[-1]][0] for qt, ids in qt_order.items()}

        for h in range(NHL):
            hp, hi = h // 2, h % 2
            po = {}
            for kc in range(KC):
                qs = cfg.qs_of(kc)
                seb = qs - 128 * kc + 512
                ps = psum.tile([128, SPAN], f32, tag="sc", bufs=3,
                               name=f"ps_s{h}_{kc}")
                lhsT = kt_sb[hp][hi * 64:(hi + 1) * 64, kc * 128:(kc + 1) * 128]
                for (o, n) in nsplit(SPAN):
                    nc.tensor.matmul(
                        ps[:, o:o + n], lhsT=lhsT,
                        rhs=qt_sb[hp][hi * 64:(hi + 1) * 64, qs + o: qs + o + n],
                        start=True, stop=True)
                et = ets.tile([128, SPAN], bf16, tag="et", name=f"et{h}_{kc}")
                nc.scalar.activation(et[:], ps[:], AF.Exp, scale=0.125)
                etb = ets.tile([128, SPAN], bf16, tag="etb", name=f"etb{h}_{kc}")
                nc.vector.tensor_mul(etb[:], et[:], eb_sb[:, seb:seb + SPAN])
                vsl = vb_sb[:, kc * VW + h * 65: kc * VW + h * 65 + 65]
                for mm_id in [i for i, m in enumerate(pv_mms) if m[0] == kc]:
                    _, qt_i, qoff, ncols, eoff = pv_mms[mm_id]
                    if qt_i not in po:
                        po[qt_i] = psum.tile([128, 512], f32, tag="one", bufs=4,
                                             name=f"po{h}_{qt_i}")
                    nc.tensor.matmul(
                        po[qt_i][0:65, qoff:qoff + ncols], lhsT=vsl,
                        rhs=etb[:, eoff:eoff + ncols],
                        start=(qt_first[qt_i] == mm_id),
                        stop=(qt_last[qt_i] == mm_id))
                for qt_i in [q for q, t in po.items() if qt_done_at[q] == kc]:
                    t = po.pop(qt_i)
                    sl = (slice(hi * 64, (hi + 1) * 64),
                          slice(qt_i * 512, (qt_i + 1) * 512))
                    nc.vector.tensor_copy(oraw_sb[hp][sl], t[0:64, :])
                    # normalize inline: engines address partitions in
                    # 32-strips, so the s row is staged at partition 0
                    s_st = stage.tile([1, 512], f32, tag="ss", name=f"ss{h}_{qt_i}")
                    nc.scalar.copy(s_st[:], t[64:65, :])
                    r_f = stage.tile([1, 512], f32, tag="rf", name=f"rf{h}_{qt_i}")
                    nc.vector.reciprocal_approx_fast(r_f[:], s_st[:])
                    r_b = stage.tile([1, 512], bf16, tag="rb", name=f"rb{h}_{qt_i}")
                    nc.vector.tensor_copy(r_b[:], r_f[:])
                    rbb = rbp.tile([128, 512], bf16, tag="rbb",
                                   name=f"rbb{h}_{qt_i}")
                    nc.gpsimd.partition_broadcast(rbb[:], r_b[:])
                    nc.vector.tensor_mul(ots_sb[hp][sl], oraw_sb[hp][sl],
                                         rbb[sl[0], :])

        # ================= Phase C: output projection =================
        for qc in range(L // 128):
            for (mo, mn) in nsplit(C):
                pf = psum.tile([128, 512], f32, tag="one", bufs=4,
                               name=f"pf{qc}_{mo}")
                for hp in range(HP):
                    nc.tensor.matmul(
                        pf[:, 0:mn],
                        lhsT=ots_sb[hp][:, qc * 128:(qc + 1) * 128],
                        rhs=wo_sb[:, hp * C + mo: hp * C + mo + mn],
                        start=(hp == 0), stop=(hp == HP - 1))
                st = ostage.tile([128, 512], f32, tag="fo", name=f"fo{qc}_{mo}")
                nc.scalar.copy(st[:, 0:mn], pf[:, 0:mn])
                nc.sync.dma_start(out[qc * 128:(qc + 1) * 128, mo:mo + mn],
                                  st[:, 0:mn])

    nc.compile()
    return nc


def host_inputs(inputs, cfg=FULL):
    """Build the 8 per-core input maps + the host-side combine constant."""
    L, C, DL, NHL = cfg.L, cfg.C, cfg.DL, cfg.NHL
    q = np.asarray(inputs["queries"], np.float32)
    k = np.asarray(inputs["keys"], np.float32)
    v = np.asarray(inputs["values"], np.float32)
    Wq = np.asarray(inputs["Wq"], np.float32)
    Wk = np.asarray(inputs["Wk"], np.float32)
    Wv = np.asarray(inputs["Wv"], np.float32)
    Wo = np.asarray(inputs["Wo"], np.float32)
    bq = np.asarray(inputs["bq"], np.float32)
    bv = np.asarray(inputs["bv"], np.float32)
    bo = np.asarray(inputs["bo"], np.float32)
    B = q.shape[0]

    bo_eff = (bo.astype(np.float64) + Wo.astype(np.float64) @ bv.astype(np.float64)
              ).astype(np.float32)

    p = np.arange(128, dtype=np.float64)[:, None]
    c = np.arange(cfg.EBW, dtype=np.float64)[None, :]
    eb = np.exp(-0.1 * np.abs(p - c + 512)).astype(BF16)

    erow = np.zeros((1, cfg.VW), BF16)
    erow[0, 64::65] = 1.0
    onesr = np.ones((1, 128), BF16)

    xT = {}
    for b in range(B):
        xT[b] = (np.ascontiguousarray(q[b].T).astype(BF16),
                 np.ascontiguousarray(k[b].T).astype(BF16),
                 np.ascontiguousarray(v[b].T).astype(BF16))

    in_maps = []
    for core in range(N_CORES):
        b, hg = core // 2, core % 2
        sl = slice(hg * DL, (hg + 1) * DL)
        wvp = np.zeros((C, cfg.VW), np.float32)
        for h in range(NHL):
            wvp[:, h * 65:h * 65 + 64] = Wv.T[:, hg * DL + h * 64: hg * DL + (h + 1) * 64]
        in_maps.append({
            "xq": xT[b][0], "xk": xT[b][1], "xv": xT[b][2],
            "wq": np.ascontiguousarray(Wq.T[:, sl]).astype(BF16),
            "wk": np.ascontiguousarray(Wk.T[:, sl]).astype(BF16),
            "wv": wvp.astype(BF16),
            "wo": np.ascontiguousarray(Wo.T[sl, :]).astype(BF16),
            "bq": np.ascontiguousarray(bq[sl][:, None]),
            "eb": eb, "erow": erow, "onesr": onesr,
        })
    return in_maps, bo_eff


_CACHED = {}


def _wait_devices_healthy(timeout_s=420):
    import time
    import jax
    import jax.numpy as jnp
    t0 = time.time()
    last = None
    while time.time() - t0 < timeout_s:
        try:
            for d in jax.devices():
                x = jax.device_put(np.ones((8, 8), np.float32), d)
                jnp.sum(x).block_until_ready()
            return
        except Exception as e:  # wedged worker recycles within a few minutes
            last = e
            time.sleep(15)
    raise RuntimeError(f"NeuronCores unhealthy after {timeout_s}s: {last}")


def kernel(**inputs):
    from concourse.bass_utils import run_bass_kernel_spmd

    cfg = FULL
    if "nc" not in _CACHED:
        _CACHED["nc"] = build_program(cfg)
    nc = _CACHED["nc"]

    in_maps, bo_eff = host_inputs(inputs, cfg)
    _wait_devices_healthy()
    try:
        res = run_bass_kernel_spmd(nc, in_maps, core_ids=list(range(N_CORES)))
    except Exception:
        _wait_devices_healthy()
        res = run_bass_kernel_spmd(nc, in_maps, core_ids=list(range(N_CORES)))
    B = np.asarray(inputs["queries"]).shape[0]
    out = np.zeros((B, cfg.L, cfg.C), np.float32)
    for b in range(B):
        out[b] = (res.results[2 * b]["out"] + res.results[2 * b + 1]["out"]
                  + bo_eff[None, :])
    return out
